# revision 5
# baseline (speedup 1.0000x reference)
"""Causal depthwise conv (kernel_size=4) on 8 TRN2 NeuronCores — fp16 hybrid.

Problem: x (4, 4096, 16, 128) f32, weight (4, 16, 128) f32,
out[b,t,h,d] = sum_k weight[k,h,d] * x[b,t-k,h,d]   (zero-pad t<0).

Sharding: tensor-parallel over heads — core c owns heads [2c, 2c+2), giving
8 streams per core (stream j = hl*BATCH + b), each a [d=128, t=4096] lane.

The kernel is HBM-DMA-bound, so all device I/O is fp16 (host casts both
ways; rel-err ~1e-3 vs the 2e-2 gate): 16.8 MB per core. DMA granularity
matters more than ring choice here: 2.1 MB transfers sustain ~405 GB/s
while 1 MB transfers measure ~275 GB/s, so streams are loaded and stored
in PAIRS (4 in-DMAs + 4 out-DMAs per core, all on the SP ring).

Compute is split so every engine stays at or below the ~41 us DMA wall:
  PE streams (0,2,4,6,7): each tap k is a matmul with a 128x128 *diagonal*
    stationary diag(w[k,h,:]) against the shifted moving slice x[d, t-k],
    accumulating all 4 taps into a PSUM bank per 512 cols; ScalarE evicts
    2048-col spans PSUM->SBUF as fp16. PE ~38 us, ACT ~29 us.
  DVE streams (1,3,5): acc = x*w0 (tensor_scalar), then 3
    scalar_tensor_tensor taps; even-offset taps pack 2x, odd offsets 1x.
    ~40 us.

Each stream ships with 4 leading zero columns (causal pad + even
alignment), so taps never read across stream boundaries and every DMA row
is a contiguous 16.4 KB run.
"""

import time

import numpy as np

import concourse.mybir as mybir
from concourse import bacc, tile
from concourse.bass_utils import run_bass_kernel_spmd

BATCH, SEQ, N_HEADS, D_HEAD = 4, 4096, 16, 128
KERNEL = 4
PAD = 4                                   # causal pad (>=KERNEL-1), even
N_CORES = 8
H_PER_CORE = N_HEADS // N_CORES           # 2
N_STREAMS = H_PER_CORE * BATCH            # 8 per core; stream j = hl*BATCH + b
S = SEQ + PAD                             # per-stream length (causal pad incl.)
N_PAIRS = N_STREAMS // 2

DVE_STREAMS = (1, 3, 5)
PE_STREAMS = (0, 2, 4, 6, 7)
BANK = 512                                # one PSUM bank in f32 cols
EVICT = 2048                              # ACT eviction span (4 banks)

F32 = mybir.dt.float32
F16 = mybir.dt.float16

PROFILE = False          # set by test.py; adds a profiled run
TRACE_KWARGS = {}
last_exec_time_ns = None
last_results = None


def _build_module(chain: bool = False, repeats: int = 1):
    """repeats>1 runs the whole kernel body that many times inside one NEFF
    (timing only). `chain` is accepted for test.py compat (unused)."""
    nc = bacc.Bacc(
        "TRN2",
        target_bir_lowering=False,
        debug=False,
        num_devices=N_CORES,
        enable_asserts=False,
    )
    x = nc.dram_tensor("x", [D_HEAD, N_STREAMS, S], F16, kind="ExternalInput").ap()
    wd = nc.dram_tensor(
        "wd", [D_HEAD, H_PER_CORE * KERNEL * D_HEAD], F16, kind="ExternalInput"
    ).ap()
    ws = nc.dram_tensor("ws", [D_HEAD, H_PER_CORE * KERNEL], F32, kind="ExternalInput").ap()
    out = nc.dram_tensor("out", [D_HEAD, N_STREAMS, SEQ], F16, kind="ExternalOutput").ap()

    with tile.TileContext(nc) as tc:
        with (
            tc.tile_pool(name="wp", bufs=1) as wp,
            tc.tile_pool(name="xp", bufs=5) as xp,
            tc.tile_pool(name="op", bufs=3) as op,
            tc.psum_pool(name="pp", bufs=2) as pp,
        ):
            wdt = wp.tile([D_HEAD, H_PER_CORE * KERNEL * D_HEAD], F16)
            wst = wp.tile([D_HEAD, H_PER_CORE * KERNEL], F32)
            nc.sync.dma_start(out=wdt, in_=wd)
            nc.sync.dma_start(out=wst, in_=ws)

            def pe_stream(X, xoff, osb, ooff, hl):
                """4 matmul taps into PSUM per 512-col bank, ACT evicts
                2048-col spans into osb[:, ooff:ooff+SEQ] as fp16."""
                for half in range(SEQ // EVICT):
                    ps = pp.tile([D_HEAD, EVICT], F32, tag="ps")
                    for k in range(KERNEL):
                        wk = wdt[:, (hl * KERNEL + k) * D_HEAD : (hl * KERNEL + k + 1) * D_HEAD]
                        for c in range(EVICT // BANK):
                            base = xoff + PAD - k + half * EVICT + c * BANK
                            nc.tensor.matmul(
                                ps[:, c * BANK : (c + 1) * BANK],
                                wk,
                                X[:, base : base + BANK],
                                start=(k == 0),
                                stop=(k == KERNEL - 1),
                            )
                    nc.scalar.activation(
                        osb[:, ooff + half * EVICT : ooff + (half + 1) * EVICT],
                        ps, mybir.ActivationFunctionType.Copy, scale=1.0,
                    )

            def dve_stream(X, xoff, osb, ooff, hl):
                acc = osb[:, ooff : ooff + SEQ]
                w0 = wst[:, hl * KERNEL : hl * KERNEL + 1]
                nc.vector.tensor_scalar(
                    acc, X[:, xoff + PAD : xoff + PAD + SEQ], w0, None,
                    mybir.AluOpType.mult,
                )
                for k in range(1, KERNEL):
                    wk = wst[:, hl * KERNEL + k : hl * KERNEL + k + 1]
                    nc.vector.scalar_tensor_tensor(
                        acc, X[:, xoff + PAD - k : xoff + PAD - k + SEQ], wk, acc,
                        mybir.AluOpType.mult, mybir.AluOpType.add,
                    )

            for _r in range(repeats):
                # paired input DMAs (2.1 MB each) up front, SP ring
                xt = []
                for a in range(N_PAIRS):
                    t = xp.tile([D_HEAD, 2 * S], F16, tag="x")
                    nc.sync.dma_start(
                        out=t.rearrange("p (j t) -> p j t", j=2),
                        in_=x[:, 2 * a : 2 * a + 2, :],
                    )
                    xt.append(t)

                for a in range(N_PAIRS):
                    ot = op.tile([D_HEAD, 2 * SEQ], F16, tag="o")
                    for p in range(2):
                        j = 2 * a + p
                        hl = j // BATCH
                        fn = dve_stream if j in DVE_STREAMS else pe_stream
                        fn(xt[a], p * S, ot, p * SEQ, hl)
                    nc.sync.dma_start(
                        out=out[:, 2 * a : 2 * a + 2, :],
                        in_=ot.rearrange("p (j t) -> p j t", j=2),
                    )
    nc.compile()
    return nc


_module = None


def _get_module():
    global _module
    if _module is None:
        _module = _build_module()
    return _module


def _shard_inputs(x: np.ndarray, weight: np.ndarray):
    x16 = x.astype(np.float16)
    w16 = weight.astype(np.float16)
    in_maps = []
    for c in range(N_CORES):
        h0 = c * H_PER_CORE
        xs = x16[:, :, h0 : h0 + H_PER_CORE, :]              # (B, T, HL, D)
        xt = np.ascontiguousarray(xs.transpose(3, 2, 0, 1))  # (D, HL, B, T)
        xin = np.zeros((D_HEAD, N_STREAMS, S), dtype=np.float16)
        xin[:, :, PAD:] = xt.reshape(D_HEAD, N_STREAMS, SEQ)

        ws_ = weight[:, h0 : h0 + H_PER_CORE, :]             # (K, HL, D) f32
        warr = np.ascontiguousarray(ws_.transpose(2, 1, 0)).reshape(
            D_HEAD, H_PER_CORE * KERNEL
        ).astype(np.float32)

        wdiag = np.zeros((D_HEAD, H_PER_CORE * KERNEL * D_HEAD), dtype=np.float16)
        for hl in range(H_PER_CORE):
            for k in range(KERNEL):
                blk = wdiag[:, (hl * KERNEL + k) * D_HEAD : (hl * KERNEL + k + 1) * D_HEAD]
                np.fill_diagonal(blk, w16[k, h0 + hl, :])
        in_maps.append({"x": xin, "wd": wdiag, "ws": warr})
    return in_maps


def _unshard(results) -> np.ndarray:
    out = np.empty((BATCH, SEQ, N_HEADS, D_HEAD), dtype=np.float32)
    for c in range(N_CORES):
        h0 = c * H_PER_CORE
        o = results[c]["out"].astype(np.float32).reshape(D_HEAD, H_PER_CORE, BATCH, SEQ)
        out[:, :, h0 : h0 + H_PER_CORE, :] = o.transpose(2, 3, 1, 0)
    return out


def kernel(x: np.ndarray, weight: np.ndarray) -> np.ndarray:
    global last_exec_time_ns, last_results
    x = np.asarray(x, dtype=np.float32)
    weight = np.asarray(weight, dtype=np.float32)
    nc = _get_module()
    in_maps = _shard_inputs(x, weight)
    # The shared terminal occasionally wedges (NRT_EXEC_UNIT_UNRECOVERABLE)
    # and recovers after a pause; retry rather than fail the whole call.
    last_err = None
    for attempt in range(3):
        try:
            res = run_bass_kernel_spmd(
                nc, in_maps, list(range(N_CORES)), trace=PROFILE, **TRACE_KWARGS
            )
            break
        except Exception as e:  # noqa: BLE001 - device-transient errors
            last_err = e
            time.sleep(25 * (attempt + 1))
    else:
        raise last_err
    last_exec_time_ns = res.exec_time_ns
    last_results = res
    return _unshard(res.results)
